# revision 14
# baseline (speedup 1.0000x reference)
"""Log2Quantizer Trainium2 kernel (raw Bass, no Tile).

Math: the reference's sort/std/rank machinery is dead code (bit_token is
unconditionally overwritten with n_bits), so the computation reduces to:
    delta[b,t] = max over (h,c) of x[b,h,t,c]
    out = delta * 2^(round(log2(max(x/delta, 1e-8))))
i.e. snap x/delta to the nearest power of two in log space, rescale by delta.

Bit-trick (no transcendentals): round(log2 r) = floor(log2(r/sqrt2)) + 1:
    y   = x * isqrt2          fused with the delta max-accumulate  (DVE)
    q   = y * (1/delta)       per-token scale                      (ACT)
    p2  = bitcast_f32(bits(q) & 0x7F800000)  = 2^floor(log2 q)     (DVE)
    out = p2 * (2*delta)      exact fp32 mult                 (M2, split)
x==0 gives q=0 -> p2=+0.0 -> out=0 (reference's clamp yields delta*2^-27
~ 7e-9 there; abs err 7e-9 on the rare exact-zero input).

Sharding: data-parallel over batch dim b (8 rows -> 8 cores), no comms.
Layout: t split into TC=512-token chunks; partition dim = t-block of tt=4 so
each partition line is one contiguous 1KB run per h in DRAM. 1KB descriptors
already saturate the per-DMA-engine bus (22.5 B/ns * 16 engines ~ 360 GB/s
aggregate, the real bottleneck at 25.2 MB total traffic -> ~70us floor).

Engine pipeline (vs the all-DVE baseline at 98.6us whose DVE ran 86us).
Per-token-scalar ops are sliced into tt per-q ops whose scalar is a [128,1]
AP; measured slice costs: DVE 613ns, ACT 1013ns. The delta reduce rides the
y = x*isqrt2 pass as a max-accumulate (tensor_scalar accum_out, op1=max),
so no separate TENSOR_REDUCE pass exists (that pass ran at half rate).
  Sync:   load DMAs (own HWDGE ring), paced to <=3 in flight: an overfull
          ring makes dma_start block and throttles the transfers themselves
          (measured 260 GB/s un-paced vs 350 paced). NBUF = n_chunks =
          whole tensor double-buffered in SBUF, so loads wait on nothing.
  DVE:    maxacc slices (y + delta), reciprocal, d2, flat single-op AND,
          M2 slices 0..M2_DVE-1 (tensor_scalar_mul, scalar AP).
  ACT:    M1 (all tt slices) + M2 slices M2_DVE..tt-1 (activation-Copy,
          per-partition scale AP) + store DMA issue (own HWDGE ring).
Per chunk: DVE ~6.8us, ACT ~6.0us, both under the DMA share ~8.1us, so in
steady state the DMA rings are the pacer, not compute.

Buffers ping-pong, no in-place ops, NBUF = n_chunks so no reuse at all:
maxacc xt->wt, M1 wt->xt, AND xt->wt, M2 wt->xt, store from xt.

Sems -- every data handoff (same-engine included: engines pipeline, e.g.
an issued DMA can read an earlier op's output before its data lands)
waits on the producer's counting increment:
  load_sem:  +16 per load DMA; DVE waits 16*(ci+1) (one FIFO ring, in-order)
  scal_sem:  +2 per chunk by DVE (recip, d2); ACT M1 waits 2*(ci+1)
  m1_sem:    +1 per ACT M1 slice; DVE AND waits tt*(ci+1)
  and_sem:   +1 per chunk by DVE AND; ACT M2b waits ci+1
  m2a_sem:   +1 per DVE M2 slice;  ACT store waits M2_DVE*(ci+1)
  m2b_sem:   +1 per ACT M2 slice;  ACT store self-waits (tt-M2_DVE)*(ci+1)
  dve_sem:   DVE-internal RAW fences (maxacc -> recip -> d2)
  store_sem: +16 per store DMA; sync tail-waits 16*n_chunks (output flushed)
"""

from contextlib import ExitStack

import numpy as np

import concourse.bass as bass
import concourse.mybir as mybir
from concourse.bass_utils import run_bass_kernel_spmd

B, H, T, C = 8, 12, 4096, 64
N_CORES = 8
P = 128          # SBUF partitions
TC = 512         # tokens per chunk (pipeline granularity)

ISQRT2 = 0.7071067811865476
EXP_MASK = 0x7F800000
M2_DVE = 3       # M2 slices 0..M2_DVE-1 on DVE, the rest on ACT

_nc_cache = {}


def _build_nc():
    if "nc" in _nc_cache:
        return _nc_cache["nc"]
    f32 = mybir.dt.float32
    i32 = mybir.dt.int32
    OP = mybir.AluOpType
    AF = mybir.ActivationFunctionType

    nc = bass.Bass()
    x_in = nc.declare_dram_parameter("x", [H, T, C], f32, isOutput=False)
    y_out = nc.declare_dram_parameter("y", [H, T, C], f32, isOutput=True)

    n_chunks = T // TC
    tt = TC // P
    FREE = H * tt * C
    NBUF = n_chunks

    def src_ap(ci):
        return x_in[:, ci * TC : (ci + 1) * TC, :].rearrange(
            "h (p q) c -> p h (q c)", p=P
        )

    def dst_ap(ci):
        return y_out[:, ci * TC : (ci + 1) * TC, :].rearrange(
            "h (p q) c -> p h (q c)", p=P
        )

    with ExitStack() as ctx:
        xt = [
            ctx.enter_context(nc.sbuf_tensor(f"xt{j}", [P, FREE], f32))
            for j in range(NBUF)
        ]
        wt = [
            ctx.enter_context(nc.sbuf_tensor(f"wt{j}", [P, FREE], f32))
            for j in range(NBUF)
        ]
        delta = [
            ctx.enter_context(nc.sbuf_tensor(f"delta{j}", [P, tt], f32))
            for j in range(NBUF)
        ]
        inv = [
            ctx.enter_context(nc.sbuf_tensor(f"inv{j}", [P, tt], f32))
            for j in range(NBUF)
        ]
        d2 = [
            ctx.enter_context(nc.sbuf_tensor(f"d2_{j}", [P, tt], f32))
            for j in range(NBUF)
        ]

        load_sem = ctx.enter_context(nc.semaphore("load_sem"))
        store_sem = ctx.enter_context(nc.semaphore("store_sem"))
        scal_sem = ctx.enter_context(nc.semaphore("scal_sem"))
        m1_sem = ctx.enter_context(nc.semaphore("m1_sem"))
        and_sem = ctx.enter_context(nc.semaphore("and_sem"))
        m2a_sem = ctx.enter_context(nc.semaphore("m2a_sem"))
        m2b_sem = ctx.enter_context(nc.semaphore("m2b_sem"))
        dve_sem = ctx.enter_context(nc.semaphore("dve_sem"))

        block = ctx.enter_context(nc.Block())

        def view4(t):
            return t[:].rearrange("p (h q c) -> p h q c", h=H, c=C)

        @block.sync
        def _(sync):
            for ci in range(n_chunks):
                if ci >= 3:
                    sync.wait_ge(load_sem, 16 * (ci - 2))
                sync.dma_start(out=xt[ci][:], in_=src_ap(ci)).then_inc(
                    load_sem, 16
                )
            # output-flush guarantee before NEFF end
            sync.wait_ge(store_sem, 16 * n_chunks)

        @block.vector
        def _(vector):
            b = 0
            for ci in range(n_chunks):
                vector.wait_ge(load_sem, 16 * (ci + 1))
                # y = x*isqrt2 (xt -> wt) fused with the per-token delta
                # max-accumulate: accum_out = max over the slice's (h, c)
                for s in range(tt):
                    vector.tensor_scalar(
                        out=view4(wt[ci])[:, :, s, :],
                        in0=view4(xt[ci])[:, :, s, :],
                        scalar1=ISQRT2,
                        scalar2=None,
                        op0=OP.mult,
                        op1=OP.max,
                        accum_out=delta[ci][:, s : s + 1],
                    ).then_inc(dve_sem, 1)
                vector.wait_ge(dve_sem, b + tt)
                # NOTE: delta here is max(x*isqrt2) = true_delta*isqrt2.
                # inv = 1/(delta') = sqrt2/true_delta, and q = y*inv =
                # x*isqrt2 * sqrt2/true_delta... that's x/true_delta, WRONG
                # by the isqrt2 factor we wanted. Compensate in inv: we need
                # q = x*isqrt2/true_delta = y/true_delta = y*isqrt2/delta'.
                # So inv := isqrt2/delta' (reciprocal then scale), and
                # d2 = 2*true_delta = sqrt2*2*delta' -> scale 2*sqrt2.
                vector.reciprocal(inv[ci][:], delta[ci][:]).then_inc(dve_sem, 1)
                vector.wait_ge(dve_sem, b + tt + 1)
                vector.tensor_scalar_mul(inv[ci][:], inv[ci][:], ISQRT2).then_inc(
                    scal_sem, 1
                )
                vector.tensor_scalar_mul(
                    d2[ci][:], delta[ci][:], 2.0 * 2.0**0.5
                ).then_inc(scal_sem, 1)
                b += tt + 1
                if ci >= 1:
                    k = ci - 1
                    # AND: p2 = bits(q) & mask, xt -> wt, one flat op
                    vector.wait_ge(m1_sem, tt * ci)
                    vector.tensor_scalar(
                        out=wt[k][:].bitcast(i32),
                        in0=xt[k][:].bitcast(i32),
                        scalar1=EXP_MASK,
                        scalar2=None,
                        op0=OP.bitwise_and,
                    ).then_inc(and_sem, 1)
                    # M2 slices 0..M2_DVE-1: out = p2 * d2, wt -> xt
                    vector.wait_ge(and_sem, ci)
                    for s in range(M2_DVE):
                        vector.tensor_scalar_mul(
                            view4(xt[k])[:, :, s, :],
                            view4(wt[k])[:, :, s, :],
                            d2[k][:, s : s + 1],
                        ).then_inc(m2a_sem, 1)
            k = n_chunks - 1
            vector.wait_ge(m1_sem, tt * n_chunks)
            vector.tensor_scalar(
                out=wt[k][:].bitcast(i32),
                in0=xt[k][:].bitcast(i32),
                scalar1=EXP_MASK,
                scalar2=None,
                op0=OP.bitwise_and,
            ).then_inc(and_sem, 1)
            vector.wait_ge(and_sem, n_chunks)
            for s in range(M2_DVE):
                vector.tensor_scalar_mul(
                    view4(xt[k])[:, :, s, :],
                    view4(wt[k])[:, :, s, :],
                    d2[k][:, s : s + 1],
                ).then_inc(m2a_sem, 1)

        @block.scalar
        def _(scalar):
            def m2b_and_store(k):
                for s in range(M2_DVE, tt):
                    scalar.activation(
                        out=view4(xt[k])[:, :, s, :],
                        in_=view4(wt[k])[:, :, s, :],
                        func=AF.Copy,
                        scale=d2[k][:, s : s + 1],
                    ).then_inc(m2b_sem, 1)
                # BOTH fences are data fences: in-stream order does NOT
                # imply the DMA reads completed data (engines pipeline)
                scalar.wait_ge(m2a_sem, M2_DVE * (k + 1))
                scalar.wait_ge(m2b_sem, (tt - M2_DVE) * (k + 1))
                scalar.dma_start(out=dst_ap(k), in_=xt[k][:]).then_inc(
                    store_sem, 16
                )

            for ci in range(n_chunks):
                scalar.wait_ge(scal_sem, 2 * (ci + 1))
                # M1: q = y * inv (wt -> xt)
                for s in range(tt):
                    scalar.activation(
                        out=view4(xt[ci])[:, :, s, :],
                        in_=view4(wt[ci])[:, :, s, :],
                        func=AF.Copy,
                        scale=inv[ci][:, s : s + 1],
                    ).then_inc(m1_sem, 1)
                if ci >= 1:
                    scalar.wait_ge(and_sem, ci)
                    m2b_and_store(ci - 1)
            scalar.wait_ge(and_sem, n_chunks)
            m2b_and_store(n_chunks - 1)

    _nc_cache["nc"] = nc
    return nc


def kernel(x: np.ndarray) -> np.ndarray:
    assert x.shape == (B, H, T, C) and x.dtype == np.float32
    nc = _build_nc()
    in_maps = [{"x": np.ascontiguousarray(x[i])} for i in range(N_CORES)]
    res = run_bass_kernel_spmd(nc, in_maps, list(range(N_CORES)))
    out = np.stack([res.results[i]["y"] for i in range(N_CORES)], axis=0)
    return out


# revision 17
# speedup vs baseline: 1.0584x; 1.0584x over previous
"""Log2Quantizer Trainium2 kernel (raw Bass, no Tile).

Math: the reference's sort/std/rank machinery is dead code (bit_token is
unconditionally overwritten with n_bits), so the computation reduces to:
    delta[b,t] = max over (h,c) of x[b,h,t,c]
    out = delta * 2^(round(log2(max(x/delta, 1e-8))))
i.e. snap x/delta to the nearest power of two in log space, rescale by delta.

Bit-trick (no transcendentals): round(log2 r) = floor(log2(r/sqrt2)) + 1:
    q   = x * (isqrt2/delta)                 per-token scale (ACT, M1)
    p2  = bitcast_f32(bits(q) & 0x7F800000)  2^floor(log2 q)   (DVE, AND)
    out = p2 * (2*delta)                     exact fp32 mult   (M2, split)
x==0 gives q=0 -> p2=+0.0 -> out=0 (reference's clamp yields delta*2^-27
~ 7e-9 there; abs err 7e-9 on the rare exact-zero input).

Sharding: data-parallel over batch dim b (8 rows -> 8 cores), no comms.
Layout: t split into chunks; partition dim = t-block of tt=TC/128 so each
partition line is one contiguous tt*256B run per h in DRAM (>=512B keeps
DMA descriptors at the full 22.5 B/ns per-engine bus rate; 16 engines ~
360 GB/s aggregate is the hard bottleneck at 25.2 MB total traffic ->
~70us floor). Chunk sizes are VARIABLE: small 256-token chunks at the
start (first store enters the DMA mix ~20us earlier, so the load/store
streams share the engines for more of the run) and at the end (the tail
drains a small last store instead of a 1.6MB one).

Engine pipeline (vs the all-DVE baseline at 98.6us whose DVE ran 86us).
Per-token-scalar ops are sliced into tt per-q ops whose scalar is a
[128,1] AP; measured slice costs: DVE ~613ns, ACT ~1017ns per 512-chunk
slice.
  Sync:   load DMAs (own HWDGE ring), paced to <=3 in flight: an overfull
          ring makes dma_start block and throttles the transfers (measured
          260 GB/s un-paced vs 350 paced). One SBUF buffer pair per chunk
          (whole tensor resident), so loads wait on nothing else.
  DVE:    2-stage max-reduce (contiguous-X over c at 2 elem/cyc, then the
          tiny strided reduce over h), reciprocal, 2 tiny per-token ops,
          flat single-op AND, M2 slices 0..m2d-1.
  ACT:    M1 (all tt slices) + M2 slices m2d..tt-1 (activation-Copy,
          per-partition scale AP) + store DMA issue (own HWDGE ring).
Per 512-chunk: DVE ~7.5us, ACT ~7.1us, within the ~8.1us/chunk DMA share,
so in steady state the DMA rings are the pacer, not compute.

Buffers ping-pong, no in-place ops, one pair per chunk so no reuse:
M1 xt->wt, AND wt->xt, M2 xt->wt, store from wt.

Sems -- every data handoff (same-engine included: engines pipeline, e.g.
an issued DMA can read an earlier op's output before its data lands)
waits on the producer's counting increment (cumulative over chunks since
slice counts vary):
  load_sem:  +16 per load DMA; DVE waits 16*(ci+1) (one FIFO ring, in-order)
  scal_sem:  +2 per chunk by DVE (inv', d2); ACT M1 waits 2*(ci+1)
  m1_sem:    +1 per ACT M1 slice; DVE AND waits cum_m1(ci)
  and_sem:   +1 per chunk by DVE AND; ACT M2b waits ci+1
  m2a_sem:   +1 per DVE M2 slice;  ACT store waits cum_m2a(ci)
  m2b_sem:   +1 per ACT M2 slice;  ACT store self-waits cum_m2b(ci)
  dve_sem:   DVE-internal RAW fences (reduce1->reduce2->recip->tinies)
  store_sem: +16 per store DMA; sync tail-waits 16*n_chunks (output flushed)
"""

from contextlib import ExitStack

import numpy as np

import concourse.bass as bass
import concourse.mybir as mybir
from concourse.bass_utils import run_bass_kernel_spmd

B, H, T, C = 8, 12, 4096, 64
N_CORES = 8
P = 128          # SBUF partitions

# chunk sizes in tokens; sum must be T. Small chunks at both ends.
CHUNKS = [256, 256, 512, 512, 512, 512, 512, 512, 256, 256]
assert sum(CHUNKS) == T

ISQRT2 = 0.7071067811865476
EXP_MASK = 0x7F800000

_nc_cache = {}


def _build_nc():
    if "nc" in _nc_cache:
        return _nc_cache["nc"]
    f32 = mybir.dt.float32
    i32 = mybir.dt.int32
    OP = mybir.AluOpType
    AF = mybir.ActivationFunctionType

    nc = bass.Bass()
    x_in = nc.declare_dram_parameter("x", [H, T, C], f32, isOutput=False)
    y_out = nc.declare_dram_parameter("y", [H, T, C], f32, isOutput=True)

    n_chunks = len(CHUNKS)
    offs = [sum(CHUNKS[:i]) for i in range(n_chunks)]
    tts = [tc // P for tc in CHUNKS]
    m2d = [tt // 2 for tt in tts]          # M2 slices on DVE per chunk

    def cum(xs, i):
        # total of xs[0..i] inclusive
        return sum(xs[: i + 1])

    def src_ap(ci):
        return x_in[:, offs[ci] : offs[ci] + CHUNKS[ci], :].rearrange(
            "h (p q) c -> p h (q c)", p=P
        )

    def dst_ap(ci):
        return y_out[:, offs[ci] : offs[ci] + CHUNKS[ci], :].rearrange(
            "h (p q) c -> p h (q c)", p=P
        )

    with ExitStack() as ctx:
        xt = [
            ctx.enter_context(
                nc.sbuf_tensor(f"xt{j}", [P, H * tts[j] * C], f32)
            )
            for j in range(n_chunks)
        ]
        wt = [
            ctx.enter_context(
                nc.sbuf_tensor(f"wt{j}", [P, H * tts[j] * C], f32)
            )
            for j in range(n_chunks)
        ]
        red = ctx.enter_context(nc.sbuf_tensor("red", [P, H * max(tts)], f32))
        delta = [
            ctx.enter_context(nc.sbuf_tensor(f"delta{j}", [P, tts[j]], f32))
            for j in range(n_chunks)
        ]
        inv = [
            ctx.enter_context(nc.sbuf_tensor(f"inv{j}", [P, tts[j]], f32))
            for j in range(n_chunks)
        ]
        d2 = [
            ctx.enter_context(nc.sbuf_tensor(f"d2_{j}", [P, tts[j]], f32))
            for j in range(n_chunks)
        ]

        load_sem = ctx.enter_context(nc.semaphore("load_sem"))
        store_sem = ctx.enter_context(nc.semaphore("store_sem"))
        scal_sem = ctx.enter_context(nc.semaphore("scal_sem"))
        m1_sem = ctx.enter_context(nc.semaphore("m1_sem"))
        and_sem = ctx.enter_context(nc.semaphore("and_sem"))
        m2a_sem = ctx.enter_context(nc.semaphore("m2a_sem"))
        m2b_sem = ctx.enter_context(nc.semaphore("m2b_sem"))
        dve_sem = ctx.enter_context(nc.semaphore("dve_sem"))

        block = ctx.enter_context(nc.Block())

        def view4(t, ci):
            return t[:].rearrange("p (h q c) -> p h q c", h=H, c=C)

        @block.sync
        def _(sync):
            for ci in range(n_chunks):
                if ci >= 3:
                    sync.wait_ge(load_sem, 16 * (ci - 2))
                sync.dma_start(out=xt[ci][:], in_=src_ap(ci)).then_inc(
                    load_sem, 16
                )
            # output-flush guarantee before NEFF end
            sync.wait_ge(store_sem, 16 * n_chunks)

        @block.vector
        def _(vector):
            def and_m2a(k):
                # AND: p2 = bits(q) & mask, wt -> xt, one flat op
                vector.wait_ge(m1_sem, cum(tts, k))
                vector.tensor_scalar(
                    out=xt[k][:].bitcast(i32),
                    in0=wt[k][:].bitcast(i32),
                    scalar1=EXP_MASK,
                    scalar2=None,
                    op0=OP.bitwise_and,
                ).then_inc(and_sem, 1)
                # M2 slices 0..m2d-1: out = p2 * d2, xt -> wt
                vector.wait_ge(and_sem, k + 1)
                for s in range(m2d[k]):
                    vector.tensor_scalar_mul(
                        view4(wt[k], k)[:, :, s, :],
                        view4(xt[k], k)[:, :, s, :],
                        d2[k][:, s : s + 1],
                    ).then_inc(m2a_sem, 1)

            b = 0
            for ci in range(n_chunks):
                tt = tts[ci]
                xt3 = xt[ci][:].rearrange("p (hq c) -> p hq c", c=C)
                redv = red[:, : H * tt]
                vector.wait_ge(load_sem, 16 * (ci + 1))
                # delta = max over (h, c) in two stages: contiguous X over
                # c, then the tiny strided reduce over h
                vector.reduce_max(
                    out=redv, in_=xt3, axis=mybir.AxisListType.X
                ).then_inc(dve_sem, 1)
                vector.wait_ge(dve_sem, b + 1)
                vector.reduce_max(
                    out=delta[ci][:],
                    in_=redv.rearrange("p (h q) -> p q h", h=H),
                    axis=mybir.AxisListType.X,
                ).then_inc(dve_sem, 1)
                vector.wait_ge(dve_sem, b + 2)
                vector.reciprocal(inv[ci][:], delta[ci][:]).then_inc(dve_sem, 1)
                vector.wait_ge(dve_sem, b + 3)
                # inv' = isqrt2/delta (M1 scale); d2 = 2*delta (M2 scale)
                vector.tensor_scalar_mul(inv[ci][:], inv[ci][:], ISQRT2).then_inc(
                    scal_sem, 1
                )
                vector.tensor_scalar_mul(d2[ci][:], delta[ci][:], 2.0).then_inc(
                    scal_sem, 1
                )
                b += 3
                if ci >= 1:
                    and_m2a(ci - 1)
            and_m2a(n_chunks - 1)

        @block.scalar
        def _(scalar):
            def m2b_and_store(k):
                # M2 slices m2d..tt-1: out = p2 * d2, xt -> wt
                for s in range(m2d[k], tts[k]):
                    scalar.activation(
                        out=view4(wt[k], k)[:, :, s, :],
                        in_=view4(xt[k], k)[:, :, s, :],
                        func=AF.Copy,
                        scale=d2[k][:, s : s + 1],
                    ).then_inc(m2b_sem, 1)
                # BOTH fences are data fences: in-stream order does NOT
                # imply the DMA reads completed data (engines pipeline).
                scalar.wait_ge(m2a_sem, cum(m2d, k))
                scalar.wait_ge(
                    m2b_sem, cum([tts[i] - m2d[i] for i in range(n_chunks)], k)
                )
                scalar.dma_start(out=dst_ap(k), in_=wt[k][:]).then_inc(
                    store_sem, 16
                )

            for ci in range(n_chunks):
                scalar.wait_ge(scal_sem, 2 * (ci + 1))
                # M1: q = x * inv' (xt -> wt)
                for s in range(tts[ci]):
                    scalar.activation(
                        out=view4(wt[ci], ci)[:, :, s, :],
                        in_=view4(xt[ci], ci)[:, :, s, :],
                        func=AF.Copy,
                        scale=inv[ci][:, s : s + 1],
                    ).then_inc(m1_sem, 1)
                if ci >= 1:
                    scalar.wait_ge(and_sem, ci)
                    m2b_and_store(ci - 1)
            scalar.wait_ge(and_sem, n_chunks)
            m2b_and_store(n_chunks - 1)

    _nc_cache["nc"] = nc
    return nc


def kernel(x: np.ndarray) -> np.ndarray:
    assert x.shape == (B, H, T, C) and x.dtype == np.float32
    nc = _build_nc()
    in_maps = [{"x": np.ascontiguousarray(x[i])} for i in range(N_CORES)]
    res = run_bass_kernel_spmd(nc, in_maps, list(range(N_CORES)))
    out = np.stack([res.results[i]["y"] for i in range(N_CORES)], axis=0)
    return out
